# revision 1
# baseline (speedup 1.0000x reference)
"""DGCNN (nn_DGCNN_39384850104582) on 8 Trainium2 NeuronCores.

Data-parallel over the batch (point-cloud) axis: each of the 8 cores runs the
full kNN/EdgeConv backbone for one cloud; the tiny classifier head (whose
BatchNorm needs cross-batch stats) is computed after gathering the per-cloud
pooled features.

Self-contained: hardcodes shapes from the problem spec (B=8, N=1024, K=20).
"""
import numpy as np

K = 20
EPS = 1e-5

_compiled = {}


def _build():
    import jax
    import jax.numpy as jnp
    from jax.sharding import Mesh, PartitionSpec as P
    from jax.experimental.shard_map import shard_map

    devs = np.array(jax.devices()[:8])
    mesh = Mesh(devs, ("b",))

    def edge_conv(x, wa, ba, wb, bb):
        # x: [N, C]
        sq = jnp.sum(x * x, axis=-1)
        d2 = sq[:, None] + sq[None, :] - 2.0 * (x @ x.T)
        idx = jax.lax.top_k(-d2, K)[1]
        xj = x[idx]
        xi = jnp.broadcast_to(x[:, None, :], xj.shape)
        e = jnp.concatenate([xi, xj - xi], -1)
        h = jax.nn.relu(e @ wa + ba) @ wb + bb
        return jnp.max(h, axis=1)

    def backbone(pos, w1a, b1a, w1b, b1b, w2a, b2a, w2b, b2b,
                 w3a, b3a, w3b, b3b, w4a, b4a, w4b, b4b, lin1_w, lin1_b):
        # pos: [1, N, 3] (this core's shard)
        x = pos[0]
        x1 = edge_conv(x, w1a, b1a, w1b, b1b)
        x2 = edge_conv(x1, w2a, b2a, w2b, b2b)
        x3 = edge_conv(x2, w3a, b3a, w3b, b3b)
        x4 = edge_conv(x3, w4a, b4a, w4b, b4b)
        xcat = jnp.concatenate([x1, x2, x3, x4], axis=-1)
        xpool = jnp.max(xcat, axis=0)
        h = xpool @ lin1_w + lin1_b
        return h[None, :]

    def full(pos, w1a, b1a, w1b, b1b, w2a, b2a, w2b, b2b,
             w3a, b3a, w3b, b3b, w4a, b4a, w4b, b4b,
             lin1_w, lin1_b, bn_g, bn_b, lin2_w, lin2_b):
        h = shard_map(
            backbone,
            mesh=mesh,
            in_specs=(P("b"),) + (P(),) * 18,
            out_specs=P("b"),
            check_rep=False,
        )(pos, w1a, b1a, w1b, b1b, w2a, b2a, w2b, b2b,
          w3a, b3a, w3b, b3b, w4a, b4a, w4b, b4b, lin1_w, lin1_b)
        mu = jnp.mean(h, axis=0)
        var = jnp.var(h, axis=0)
        hn = bn_g * (h - mu) * jax.lax.rsqrt(var + EPS) + bn_b
        hr = jax.nn.relu(hn)
        logits = hr @ lin2_w + lin2_b
        return jax.nn.log_softmax(logits, axis=1)

    return jax.jit(full)


def kernel(**inputs) -> np.ndarray:
    import jax

    if "fn" not in _compiled:
        _compiled["fn"] = _build()
    fn = _compiled["fn"]
    order = ["pos",
             "w1a", "b1a", "w1b", "b1b", "w2a", "b2a", "w2b", "b2b",
             "w3a", "b3a", "w3b", "b3b", "w4a", "b4a", "w4b", "b4b",
             "lin1_w", "lin1_b", "bn_g", "bn_b", "lin2_w", "lin2_b"]
    args = [np.asarray(inputs[k]) for k in order]
    out = fn(*args)
    return np.asarray(jax.device_get(out)).astype(np.float32)

